# revision 2
# baseline (speedup 1.0000x reference)
"""Trainium2 Bass kernel for nn_LinearE2V_wo_global (gnn_message_passing).

Strategy (8 cores, batch B=8 -> one batch per core, pure data parallel):

Per-batch computation (N=E=2048 tokens, D=H=128):
  stage E1/V1: x += MLP1(LN1(x) + pe)        (edges use pe1[edge_orders])
  agg = incidence @ x_e_masked;  x1 = x_v + agg/(1+sn)
  stage P2:    x1 += MLP2(LN2(x1) + pe2_s1)
  stage P3:    x  += MLP3(LN3(x)) + bias_b;  out = node_mask * x

Device design (token-major activations [tok=128p, d=128f]):
  - LN via bn_stats/bn_aggr; normalize folded into one ACT op per subtile
    (scale=rstd, bias=-mu*rstd per-partition).
  - LN gain g folded into W1 (host: W1g = g[:,None]*W1). LN bias + pe + b1
    folded into a per-token bias row: for nodes a constant column added at
    the relu (ACT bias); for edges a host-gathered table b1effT[h,e] added
    on DVE before the relu.
  - MLP matmuls in float32r hi/lo split (3 chains, 1 cyc/row each,
    ~2^-22 error). mlp b2 folded into the PSUM-evict ACT bias; bias_b
    folded into mlp3_b2.
  - Aggregation: incidence sent as bf16 (0/1 exact; halves HBM traffic),
    edge_mask folded into incidence on host. x_e in bf16 hi/lo split
    (2 chains, exact to ~2^-17). out = aggT [d,n], PE-transposed back.
  - node_mask applied once at the very end (per-token ops never leak
    across tokens, so masked rows are garbage-until-final-mask).
"""
import sys

sys.path.insert(0, "/opt/trn_rl_repo")

import numpy as np
import ml_dtypes

import concourse.bass as bass
import concourse.bacc as bacc
import concourse.tile as tile
from concourse import mybir
from concourse.bass_utils import run_bass_kernel_spmd
from concourse.masks import make_identity

B, N, E, D, H = 8, 2048, 2048, 128, 128
NBLK = N // 128        # 16 token blocks of 128
NGRP = NBLK // 4       # 4 groups of 512 tokens
LN_EPS = 1e-5

F32 = mybir.dt.float32
F32R = mybir.dt.float32r
BF16 = mybir.dt.bfloat16

AF = mybir.ActivationFunctionType

_CACHE = {}


def _build_nc():
    nc = bacc.Bacc()

    # ---------------- dram params ----------------
    xv_p = nc.declare_dram_parameter("xv", [N, D], F32, isOutput=False)
    xe_p = nc.declare_dram_parameter("xe", [E, D], F32, isOutput=False)
    inc_p = nc.declare_dram_parameter("incTm", [E, N], BF16, isOutput=False)
    b1e_p = nc.declare_dram_parameter("b1effT", [H, E], F32, isOutput=False)
    invsn_p = nc.declare_dram_parameter("invsn", [128, NBLK], F32, isOutput=False)
    nm_p = nc.declare_dram_parameter("nm", [128, NBLK], F32, isOutput=False)
    w1_p = [nc.declare_dram_parameter(f"w1g{s}", [D, H], F32, isOutput=False)
            for s in (1, 2, 3)]
    w2_p = [nc.declare_dram_parameter(f"w2{s}", [H, D], F32, isOutput=False)
            for s in (1, 2, 3)]
    # columns: relu biases (stages 1v,2,3) and b2 evict biases (stages 1,2,3)
    cv_p = [nc.declare_dram_parameter(f"cv{s}", [H, 1], F32, isOutput=False)
            for s in (1, 2, 3)]
    b2_p = [nc.declare_dram_parameter(f"b2c{s}", [D, 1], F32, isOutput=False)
            for s in (1, 2, 3)]
    out_p = nc.declare_dram_parameter("out", [N, D], F32, isOutput=True)

    xv_r = xv_p.rearrange("(blk p) d -> blk p d", p=128)
    xe_r = xe_p.rearrange("(blk p) d -> blk p d", p=128)
    inc_r = inc_p.rearrange("(c k) n -> c k n", k=128)
    out_r = out_p.rearrange("(blk p) d -> blk p d", p=128)

    with tile.TileContext(nc) as tc:
        with (
            tc.tile_pool(name="consts", bufs=1) as consts,
            tc.tile_pool(name="acts", bufs=1) as acts,
            tc.tile_pool(name="stats", bufs=2) as stats_pool,
            tc.tile_pool(name="work", bufs=2) as work,
            tc.tile_pool(name="incs", bufs=20) as incs,
            tc.tile_pool(name="pT", bufs=2, space="PSUM") as pT,
            tc.tile_pool(name="pH", bufs=2, space="PSUM") as pH,
            tc.tile_pool(name="pO", bufs=2, space="PSUM") as pO,
            tc.tile_pool(name="pB", bufs=2, space="PSUM") as pB,
        ):
            # ---------------- constants ----------------
            ident = consts.tile([128, 128], F32, tag="ident")
            make_identity(nc, ident)
            eps = consts.tile([128, 1], F32, tag="eps")
            nc.vector.memset(eps, LN_EPS)

            w_raw = []
            for s in range(3):
                a = consts.tile([D, H], F32, tag=f"w1r{s}")
                nc.sync.dma_start(out=a, in_=w1_p[s][:, :])
                b = consts.tile([H, D], F32, tag=f"w2r{s}")
                nc.sync.dma_start(out=b, in_=w2_p[s][:, :])
                w_raw.append((a, b))
            # device-side f32r hi/lo splits of the weights
            w1_hi, w1_lo, w2_hi, w2_lo = [], [], [], []
            for s in range(3):
                for (raw, his, los, tg) in (
                    (w_raw[s][0], w1_hi, w1_lo, f"w1"),
                    (w_raw[s][1], w2_hi, w2_lo, f"w2"),
                ):
                    hi = consts.tile([128, 128], F32R, tag=f"{tg}hi{s}")
                    lo = consts.tile([128, 128], F32R, tag=f"{tg}lo{s}")
                    nc.scalar.copy(hi, raw)
                    nc.vector.tensor_sub(lo, raw, hi)
                    his.append(hi)
                    los.append(lo)

            cv = []
            b2c = []
            for s in range(3):
                t = consts.tile([H, 1], F32, tag=f"cv{s}")
                nc.sync.dma_start(out=t, in_=cv_p[s][:, :])
                cv.append(t)
                t2 = consts.tile([D, 1], F32, tag=f"b2c{s}")
                nc.sync.dma_start(out=t2, in_=b2_p[s][:, :])
                b2c.append(t2)
            invsn = consts.tile([128, NBLK], F32, tag="invsn")
            nc.sync.dma_start(out=invsn, in_=invsn_p[:, :])
            nm = consts.tile([128, NBLK], F32, tag="nm")
            nc.sync.dma_start(out=nm, in_=nm_p[:, :])

            # ---------------- activations resident in SBUF ----------------
            xv_tok = acts.tile([128, NBLK, 128], F32, tag="xv")
            xe_tok = acts.tile([128, NBLK, 128], F32, tag="xe")
            for blk in range(NBLK):
                nc.sync.dma_start(out=xv_tok[:, blk, :], in_=xv_r[blk])
                nc.sync.dma_start(out=xe_tok[:, blk, :], in_=xe_r[blk])
            b1effT = acts.tile([128, NGRP, 512], F32, tag="b1effT")
            for g in range(NGRP):
                nc.sync.dma_start(out=b1effT[:, g, :],
                                  in_=b1e_p[:, g * 512:(g + 1) * 512])

            xe_new = acts.tile([128, NBLK, 128], F32, tag="xe_new")
            xe_hi = acts.tile([128, NBLK, 128], BF16, tag="xe_hi")
            xe_lo = acts.tile([128, NBLK, 128], BF16, tag="xe_lo")
            xv1 = acts.tile([128, NBLK, 128], F32, tag="xv1")
            x1 = acts.tile([128, NBLK, 128], F32, tag="x1")
            x2 = acts.tile([128, NBLK, 128], F32, tag="x2")
            o_sb = acts.tile([128, NBLK, 128], F32, tag="o_sb")

            # incidence stream tiles: [e-chunk c 128, n-block nb 512] bf16
            inc_tiles = {}
            for nb in range(4):
                for c in range(NBLK):
                    t = incs.tile([128, 512], BF16, tag="inc")
                    nc.sync.dma_start(out=t, in_=inc_r[c][:, nb * 512:(nb + 1) * 512])
                    inc_tiles[(c, nb)] = t

            # ---------------- MLP phase ----------------
            def mlp_phase(x_in, out_tile, s, is_edge, final_mask=False):
                """out_tile = x_in + MLP_s(LN_s(x_in) + pe)  [stage-folded]"""
                # pass A: stats for all 16 blocks
                mv = stats_pool.tile([128, NBLK, 2], F32, tag="mv")
                for blk in range(NBLK):
                    st = stats_pool.tile([128, 6], F32, tag="bnst")
                    nc.vector.bn_stats(out=st, in_=x_in[:, blk, :])
                    nc.vector.bn_aggr(out=mv[:, blk, :], in_=st)
                # pass B: rstd = 1/sqrt(var+eps), negms = -mu*rstd
                rstd = stats_pool.tile([128, NBLK], F32, tag="rstd")
                nc.scalar.activation(out=rstd, in_=mv[:, :, 1], func=AF.Sqrt,
                                     bias=eps, scale=1.0)
                nc.vector.reciprocal(out=rstd, in_=rstd)
                negmu = stats_pool.tile([128, NBLK], F32, tag="negmu")
                nc.scalar.activation(out=negmu, in_=mv[:, :, 0], func=AF.Copy,
                                     scale=-1.0)
                negms = stats_pool.tile([128, NBLK], F32, tag="negms")
                nc.vector.tensor_mul(negms, negmu, rstd)

                # pass C: per 512-token group
                for g in range(NGRP):
                    xn = work.tile([128, 4, 128], F32, tag="xn")
                    psT = pT.tile([128, 512], F32, tag="psT")
                    for i in range(4):
                        blk = 4 * g + i
                        nc.scalar.activation(
                            out=xn[:, i, :], in_=x_in[:, blk, :], func=AF.Identity,
                            bias=negms[:, blk:blk + 1], scale=rstd[:, blk:blk + 1])
                        nc.tensor.transpose(psT[:, i * 128:(i + 1) * 128],
                                            xn[:, i, :], ident)
                    hiT = work.tile([128, 512], F32R, tag="hiT")
                    loT = work.tile([128, 512], F32R, tag="loT")
                    nc.scalar.copy(hiT, psT)
                    nc.vector.tensor_sub(loT, psT, hiT)

                    psH = pH.tile([128, 512], F32, tag="psH")
                    nc.tensor.matmul(psH, w1_hi[s], hiT, start=True, stop=False)
                    nc.tensor.matmul(psH, w1_hi[s], loT, start=False, stop=False)
                    nc.tensor.matmul(psH, w1_lo[s], hiT, start=False, stop=True)

                    r32 = work.tile([128, 512], F32, tag="r32")
                    if is_edge:
                        rpre = work.tile([128, 512], F32, tag="rpre")
                        nc.vector.tensor_add(rpre, psH, b1effT[:, g, :])
                        nc.scalar.activation(out=r32, in_=rpre, func=AF.Relu)
                    else:
                        nc.scalar.activation(out=r32, in_=psH, func=AF.Relu,
                                             bias=cv[s], scale=1.0)
                    rhi = work.tile([128, 512], F32R, tag="rhi")
                    rlo = work.tile([128, 512], F32R, tag="rlo")
                    nc.scalar.copy(rhi, r32)
                    nc.vector.tensor_sub(rlo, r32, rhi)

                    psO = pO.tile([128, 512], F32, tag="psO")
                    nc.tensor.matmul(psO, w2_hi[s], rhi, start=True, stop=False)
                    nc.tensor.matmul(psO, w2_hi[s], rlo, start=False, stop=False)
                    nc.tensor.matmul(psO, w2_lo[s], rhi, start=False, stop=True)

                    s2 = work.tile([128, 512], F32, tag="s2")
                    nc.scalar.activation(out=s2, in_=psO, func=AF.Identity,
                                         bias=b2c[s], scale=1.0)
                    psB = pB.tile([128, 4, 128], F32, tag="psB")
                    for i in range(4):
                        nc.tensor.transpose(psB[:, i, :],
                                            s2[:, i * 128:(i + 1) * 128], ident)
                    if final_mask:
                        tmp = work.tile([128, 4, 128], F32, tag="fm")
                        nc.vector.tensor_add(tmp, psB,
                                             x_in[:, 4 * g:4 * g + 4, :])
                        for i in range(4):
                            blk = 4 * g + i
                            nc.vector.tensor_scalar_mul(
                                out=out_tile[:, blk, :], in0=tmp[:, i, :],
                                scalar1=nm[:, blk:blk + 1])
                    else:
                        nc.vector.tensor_add(out_tile[:, 4 * g:4 * g + 4, :],
                                             psB, x_in[:, 4 * g:4 * g + 4, :])

            # ---------------- pipeline ----------------
            mlp_phase(xe_tok, xe_new, 0, is_edge=True)
            # bf16 hi/lo split of updated edge features (agg lhsT)
            for g in range(NGRP):
                sl = slice(4 * g, 4 * g + 4)
                nc.scalar.copy(xe_hi[:, sl, :], xe_new[:, sl, :])
                nc.vector.tensor_sub(xe_lo[:, sl, :], xe_new[:, sl, :],
                                     xe_hi[:, sl, :])

            mlp_phase(xv_tok, xv1, 0, is_edge=False)

            # aggregation: aggT[d, n] = sum_e xe[e, d] * incTm[e, n]
            for nb in range(4):
                psA = pH.tile([128, 512], F32, tag="psH")
                for c in range(NBLK):
                    t = inc_tiles[(c, nb)]
                    nc.tensor.matmul(psA, xe_hi[:, c, :], t,
                                     start=(c == 0), stop=False)
                    nc.tensor.matmul(psA, xe_lo[:, c, :], t,
                                     start=False, stop=(c == NBLK - 1))
                aT = work.tile([128, 512], F32, tag="aT")
                nc.scalar.copy(aT, psA)
                psB2 = pB.tile([128, 4, 128], F32, tag="psB")
                for j in range(4):
                    nc.tensor.transpose(psB2[:, j, :],
                                        aT[:, j * 128:(j + 1) * 128], ident)
                for j in range(4):
                    blk = 4 * nb + j
                    tj = work.tile([128, 128], F32, tag="aggj")
                    nc.scalar.activation(out=tj, in_=psB2[:, j, :], func=AF.Copy,
                                         scale=invsn[:, blk:blk + 1])
                    nc.vector.tensor_add(x1[:, blk, :], tj, xv1[:, blk, :])

            mlp_phase(x1, x2, 1, is_edge=False)
            mlp_phase(x2, o_sb, 2, is_edge=False, final_mask=True)

            for blk in range(NBLK):
                nc.sync.dma_start(out=out_r[blk], in_=o_sb[:, blk, :])

    nc.finalize()
    return nc


def _host_prep(x_v, x_e, incidence, suffix_normalizer, edge_orders,
               node_mask, edge_mask, pe1_table, pe2_table, bias_b,
               mlp1_W1, mlp1_b1, mlp1_W2, mlp1_b2,
               mlp2_W1, mlp2_b1, mlp2_W2, mlp2_b2,
               mlp3_W1, mlp3_b1, mlp3_W2, mlp3_b2,
               norm1_g, norm1_b, norm2_g, norm2_b, norm3_g, norm3_b):
    f32 = np.float32
    x_v = np.ascontiguousarray(np.asarray(x_v, dtype=f32))
    x_e = np.ascontiguousarray(np.asarray(x_e, dtype=f32))
    incidence = np.asarray(incidence, dtype=f32)
    sn = np.asarray(suffix_normalizer, dtype=f32)
    eo = np.asarray(edge_orders).astype(np.int64)
    nmk = np.asarray(node_mask).astype(f32)
    emk = np.asarray(edge_mask).astype(f32)
    pe1 = np.asarray(pe1_table, dtype=f32)
    pe2 = np.asarray(pe2_table, dtype=f32)
    bias_b = np.asarray(bias_b, dtype=f32)

    Ws = [(np.asarray(mlp1_W1, f32), np.asarray(mlp1_b1, f32),
           np.asarray(mlp1_W2, f32), np.asarray(mlp1_b2, f32),
           np.asarray(norm1_g, f32), np.asarray(norm1_b, f32)),
          (np.asarray(mlp2_W1, f32), np.asarray(mlp2_b1, f32),
           np.asarray(mlp2_W2, f32), np.asarray(mlp2_b2, f32),
           np.asarray(norm2_g, f32), np.asarray(norm2_b, f32)),
          (np.asarray(mlp3_W1, f32), np.asarray(mlp3_b1, f32),
           np.asarray(mlp3_W2, f32), np.asarray(mlp3_b2, f32),
           np.asarray(norm3_g, f32), np.asarray(norm3_b, f32))]

    shared = {}
    for s, (W1, b1, W2, b2, g, b) in enumerate(Ws, start=1):
        shared[f"w1g{s}"] = np.ascontiguousarray(g[:, None] * W1)
        shared[f"w2{s}"] = np.ascontiguousarray(W2)
    # per-k bias table for stage 1: (norm1_b + pe1_k) @ W1 + b1
    W1_1, b1_1 = Ws[0][0], Ws[0][1]
    tab1 = (Ws[0][5][None, :] + pe1) @ W1_1 + b1_1          # [9, H]
    shared["cv1"] = np.ascontiguousarray(tab1[1][:, None])   # nodes: k=1
    shared["cv2"] = np.ascontiguousarray(
        (((Ws[1][5] + pe2[1]) @ Ws[1][0]) + Ws[1][1])[:, None])
    shared["cv3"] = np.ascontiguousarray(
        ((Ws[2][5] @ Ws[2][0]) + Ws[2][1])[:, None])
    shared["b2c1"] = np.ascontiguousarray(Ws[0][3][:, None])
    shared["b2c2"] = np.ascontiguousarray(Ws[1][3][:, None])
    shared["b2c3"] = np.ascontiguousarray((Ws[2][3] + bias_b[0])[:, None])

    in_maps = []
    for bidx in range(B):
        m = dict(shared)
        m["xv"] = x_v[bidx]
        m["xe"] = x_e[bidx]
        # fold edge mask into the (transposed) incidence; bf16 is exact on 0/1
        incT = incidence[bidx].T * emk[bidx][:, None]
        m["incTm"] = np.ascontiguousarray(incT).astype(ml_dtypes.bfloat16)
        m["b1effT"] = np.ascontiguousarray(tab1[eo[bidx]].T)   # [H, E]
        m["invsn"] = np.ascontiguousarray(
            (1.0 / (1.0 + sn[bidx])).reshape(NBLK, 128).T)
        m["nm"] = np.ascontiguousarray(nmk[bidx].reshape(NBLK, 128).T)
        in_maps.append(m)
    return in_maps


def _get_nc():
    if "nc" not in _CACHE:
        _CACHE["nc"] = _build_nc()
    return _CACHE["nc"]


def run(in_maps, trace=False, tmpdir=None):
    nc = _get_nc()
    return run_bass_kernel_spmd(nc, in_maps, list(range(B)), trace=trace, tmpdir=tmpdir)


def kernel(**inputs) -> np.ndarray:
    in_maps = _host_prep(**inputs)
    res = run(in_maps)
    out = np.stack([res.results[i]["out"] for i in range(B)], axis=0)
    return out.astype(np.float32)
